# revision 2
# baseline (speedup 1.0000x reference)
"""KAN layer (identity edges) Trainium2 kernel.

output[b, o] = sum_i x[b, i]  for all o  -- row-sum broadcast to (B, 1024).

Data-parallel over 8 NeuronCores: each core gets 8192 rows of x
(65536 x 1024 f32), computes row sums on the Vector engine, broadcasts
across the feature dim on-chip, and DMAs the full (8192, 1024) shard out.

Layout: partition p owns 64 consecutive DRAM rows (rearrange
"(p n) d -> p n d"), so each DMA moves R*4KB contiguous bytes per
partition.

Perf notes (HW-traced):
- Loads go on the SP HWDGE ring, stores on the ACT HWDGE ring; the two
  rings share the ~435 GB/s SBUF-AXI/SDMA budget via per-packet
  round-robin, so a 1:1 queue split keeps read and write streams
  balanced (64 MB/core total -> ~147 us DMA floor uncontended).
- The first/last iterations use smaller tiles (ramp) so the write
  stream starts earlier and the tail write runs shorter solo.
- Compute (reduce ~8.7us + bcast copy ~4.4us per 8-row tile on DVE)
  stays fully hidden under DMA.
"""

import numpy as np

import concourse.tile as tile
from concourse import bacc, mybir
from concourse.bass_utils import run_bass_kernel_spmd

N_CORES = 8
BATCH = 65536
FEAT = 1024
ROWS = BATCH // N_CORES        # 8192 rows per core
P = 128                        # SBUF partitions
ROWS_PER_PART = ROWS // P      # 64 consecutive rows owned by each partition

R_SCHED = (2, 2, 4, 8, 8, 8, 8, 8, 8, 4, 4)
IN_BUFS = 3
OUT_BUFS = 3

_nc_cache = []


def _build():
    assert sum(R_SCHED) == ROWS_PER_PART
    nc = bacc.Bacc()
    x = nc.declare_dram_parameter("x", [ROWS, FEAT], mybir.dt.float32, isOutput=False)
    y = nc.declare_dram_parameter("y", [ROWS, FEAT], mybir.dt.float32, isOutput=True)
    xv = x[:, :].rearrange("(p n) d -> p n d", p=P)
    yv = y[:, :].rearrange("(p n) d -> p n d", p=P)

    with tile.TileContext(nc) as tc:
        with (
            tc.tile_pool(name="inp", bufs=IN_BUFS) as inp,
            tc.tile_pool(name="outp", bufs=OUT_BUFS) as outp,
            tc.tile_pool(name="sums", bufs=4) as sums_pool,
        ):
            row = 0
            for r in R_SCHED:
                t = inp.tile([P, r, FEAT], mybir.dt.float32, tag="in")
                nc.sync.dma_start(out=t[:, :, :], in_=xv[:, row : row + r, :])

                s = sums_pool.tile([P, r], mybir.dt.float32, tag="s")
                nc.vector.reduce_sum(
                    out=s[:, :], in_=t[:, :, :], axis=mybir.AxisListType.X
                )

                o = outp.tile([P, r, FEAT], mybir.dt.float32, tag="out")
                nc.vector.tensor_copy(
                    out=o[:, :, :], in_=s[:, :].to_broadcast([P, r, FEAT])
                )
                nc.scalar.dma_start(out=yv[:, row : row + r, :], in_=o[:, :, :])
                row += r
    nc.finalize()
    return nc


def _get_nc():
    if not _nc_cache:
        _nc_cache.append(_build())
    return _nc_cache[0]


def kernel(x: np.ndarray) -> np.ndarray:
    nc = _get_nc()
    x = np.ascontiguousarray(np.asarray(x), dtype=np.float32)
    shards = np.split(x, N_CORES, axis=0)
    in_maps = [{"x": s} for s in shards]
    res = run_bass_kernel_spmd(nc, in_maps, list(range(N_CORES)))
    return np.concatenate([res.results[i]["y"] for i in range(N_CORES)], axis=0)
